# revision 43
# baseline (speedup 1.0000x reference)
"""JumpAttention (channel attention, cross-swapped values) on 8 trn2 cores.

v3: 3-phase streamed schedule, pure data-parallel over B (B=8 == n_cores).

Math (per batch, per head h, hd=64):
  G_h  = q_h k_h^T (contraction over N) == Wq_h^T (x^T x) Wk_h
  attn = softmax(G / (||q|| ||k||) * temp, axis=d)
  y1+y2 = x2 @ F1 + x1 @ F2,  F_s = concat_h(Wv_h @ attn_s_h^T)

Schedule:
  Phase A: stream x1 slabs (contiguous per-partition descriptors,
           "(p t) c"): cast f16, S1 += x1^T x1, PE-transpose into
           resident xT1 (f16).
  IL1:     attn1 from S1 -> F1 (f16). rsqrt via exp(-.5 ln n) so the
           ACT engine only ever needs the {copy, ln, exp} table.
  Phase B: stream x2 slabs: cast f16 + fp8; S2 via fp8 DoubleRow
           matmuls; transpose -> xt2 (transient); y1 partial
           = x2 @ F1 -> resident y1p (f16). x2 is never stored.
           x2@F1 matmuls run two slabs behind their transposes so the
           PE never round-trips through the copy engines.
  IL2:     attn2 from S2 -> F2
  Phase C: out = y1p + x1 @ F2 (from xT1), f16 out, host upcasts.

All DMAs use the "(p t) c" token permutation (partition-major contiguous
rows, 4KB/2KB descriptors); input and output use the same permutation so
it cancels. S sums are token-order invariant.
"""

import os
import sys
from contextlib import ExitStack

import numpy as np
import ml_dtypes

for _p in ("/opt/trn_rl_repo",):
    if _p not in sys.path and os.path.isdir(_p):
        sys.path.insert(0, _p)

import concourse.bass as bass  # noqa: E402
import concourse.tile as tile  # noqa: E402
from concourse import bacc, mybir  # noqa: E402
from concourse.bass_utils import run_bass_kernel_spmd  # noqa: E402

B, N_FULL, C = 8, 16384, 256
H, HD = 4, 64
NCORES = 8
TT = 128
SLAB = 4

F32 = mybir.dt.float32
F16 = mybir.dt.float16
FP8 = mybir.dt.float8e4
AF = mybir.ActivationFunctionType
DR = mybir.MatmulPerfMode.DoubleRow
EPS2 = 1e-24  # clamp on squared norms ~ (eps=1e-12)**2


def _build(n_tokens: int):
    nt = n_tokens // TT
    nslab = n_tokens // (TT * SLAB)
    nc = bacc.Bacc(
        "TRN2", target_bir_lowering=False, debug=False, num_devices=NCORES
    )
    x1 = nc.dram_tensor("x1", [n_tokens, C], F32, kind="ExternalInput").ap()
    x2 = nc.dram_tensor("x2", [n_tokens, C], F32, kind="ExternalInput").ap()
    wblob = nc.dram_tensor("wblob", [128, 3200], F16, kind="ExternalInput").ap()
    tmpd = nc.dram_tensor("tmpd", [128, 4], F32, kind="ExternalInput").ap()
    out = nc.dram_tensor("out", [n_tokens, C], F16, kind="ExternalOutput").ap()

    with tile.TileContext(nc) as tc, ExitStack() as ctx:
        _kernel(ctx, tc, out, x1, x2, wblob, tmpd, nt, nslab)
    nc.compile()
    return nc


def _interlude(tc, sb, wv, S_sb, lntmp_cc, F_sb):
    """One stream: S (f16 [128, 2C], chunks side by side) -> F (f16, 2 chunks).

    lntmp_cc: [128, 2] f32, per-cc ln(temperature) striped per partition.
    Only Copy/Ln/Exp are used on ACT (single function table).
    """
    nc = tc.nc
    with ExitStack() as il:
        big = il.enter_context(tc.tile_pool(name="ilbig", bufs=6, space="PSUM"))
        small = il.enter_context(tc.tile_pool(name="ilsmall", bufs=2, space="PSUM"))

        Sv = [S_sb[:, jc * C : (jc + 1) * C] for jc in range(2)]

        # -- stage A: [T_k | T_q] = S @ [Wk | Wq] --
        t_sb = {}
        for ic in range(2):
            tp = big.tile([128, 2 * C], F32, tag="ilbig", name="ilbig")
            for jc in range(2):
                nc.tensor.matmul(
                    tp[:],
                    lhsT=Sv[jc][:, ic * 128 : (ic + 1) * 128],
                    rhs=wv["wkq_sb"][jc],
                    start=(jc == 0),
                    stop=(jc == 1),
                )
            st = sb.tile([128, 2 * C], F16, tag=f"t{ic}", name=f"t{ic}")
            nc.vector.tensor_copy(st[:, 0:C], tp[:, 0:C])
            nc.vector.tensor_copy(st[:, C : 2 * C], tp[:, C : 2 * C])
            t_sb["k", ic] = st[:, 0:C]
            t_sb["q", ic] = st[:, C : 2 * C]

        # -- stage B: G = Wq^T @ T_k ; U = W * T --
        g_ps = {}
        for cc in range(2):
            g = big.tile([128, C], F32, tag="ilbig", name="ilbig")
            for ic in range(2):
                nc.tensor.matmul(
                    g[:],
                    lhsT=wv["wq_sb"][ic][:, cc * 128 : (cc + 1) * 128],
                    rhs=t_sb["k", ic],
                    start=(ic == 0),
                    stop=(ic == 1),
                )
            g_ps[cc] = g
        u_sb = {}
        for nm in ("q", "k"):
            w_sb = wv["wq_sb"] if nm == "q" else wv["wk_sb"]
            for ic in range(2):
                u = sb.tile([128, C], F16, tag=f"u{nm}{ic}", name=f"u{nm}{ic}")
                nc.vector.tensor_mul(u[:], w_sb[ic], t_sb[nm, ic])
                u_sb[nm, ic] = u

        # -- stage C: squared norms + rsqrt via exp(-.5 ln n) --
        nq_ps = {}
        for cc in range(2):
            nq = small.tile([128, 1], F32, tag="ilsmall", name="ilsmall")
            for ic in range(2):
                nc.tensor.matmul(
                    nq[:],
                    lhsT=u_sb["q", ic][:, cc * 128 : (cc + 1) * 128],
                    rhs=wv["ones_col"][:],
                    start=(ic == 0),
                    stop=(ic == 1),
                )
            nq_ps[cc] = nq
        nk = small.tile([1, C], F32, tag="ilsmall", name="ilsmall_r")
        for ic in range(2):
            nc.tensor.matmul(
                nk[:],
                lhsT=wv["ones_col"][:],
                rhs=u_sb["k", ic],
                start=(ic == 0),
                stop=(ic == 1),
            )
        # cluster all Ln ops, then all Exp ops: exactly two ACT table loads
        lq = {}
        for cc in range(2):
            l_ = sb.tile([128, 1], F32, tag=f"lq{cc}", name=f"lq{cc}")
            nc.vector.tensor_scalar_max(l_[:], nq_ps[cc][:], EPS2)
            lq[cc] = l_
        lk = sb.tile([1, C], F32, tag="lk", name="lk")
        nc.vector.tensor_scalar_max(lk[:], nk[:], EPS2)
        for cc in range(2):
            nc.scalar.activation(lq[cc][:], lq[cc][:], AF.Ln)
        nc.scalar.activation(lk[:], lk[:], AF.Ln)
        invq = {}
        for cc in range(2):
            iq = sb.tile([128, 1], F32, tag=f"invq{cc}", name=f"invq{cc}")
            # invq = exp(-.5 ln nq + ln temp) = temp / ||q||
            nc.scalar.activation(
                iq[:], lq[cc][:], AF.Exp, scale=-0.5,
                bias=lntmp_cc[:, cc : cc + 1],
            )
            invq[cc] = iq
        ik16 = sb.tile([1, C], F16, tag="invk16", name="invk16")
        nc.scalar.activation(ik16[:], lk[:], AF.Exp, scale=-0.5)

        # -- stage D: invk broadcast + logits + row max --
        bc = big.tile([128, C], F32, tag="ilbig", name="ilbig")
        nc.tensor.matmul(
            bc[:], lhsT=wv["ones_row"][:], rhs=ik16[:], start=True, stop=True
        )
        lp_sb, nm_sb = {}, {}
        for cc in range(2):
            lp = sb.tile([128, HD], F32, tag=f"lp{cc}", name=f"lp{cc}")
            for half in range(2):
                h = 2 * cc + half
                rs = slice(half * 64, (half + 1) * 64)
                cs = slice(h * 64, (h + 1) * 64)
                nc.vector.tensor_scalar_mul(
                    lp[rs, :], g_ps[cc][rs, cs], invq[cc][rs, :]
                )
                nc.vector.tensor_mul(lp[rs, :], lp[rs, :], bc[rs, cs])
            nmt = sb.tile([128, 1], F32, tag=f"nm{cc}", name=f"nm{cc}")
            nc.vector.tensor_reduce(
                nmt[:], lp[:], axis=mybir.AxisListType.X,
                op=mybir.AluOpType.max, negate=True,
            )
            lp_sb[cc] = lp
            nm_sb[cc] = nmt

        # -- stage E: Exp + normalize --
        a16 = {}
        for cc in range(2):
            pexp = sb.tile([128, HD], F32, tag=f"pexp{cc}", name=f"pexp{cc}")
            sm = sb.tile([128, 1], F32, tag=f"sm{cc}", name=f"sm{cc}")
            nc.scalar.activation(
                pexp[:], lp_sb[cc][:], AF.Exp,
                bias=nm_sb[cc][:], accum_out=sm[:],
            )
            a16[cc] = (pexp, sm)
        for cc in range(2):
            pexp, sm = a16[cc]
            nc.vector.reciprocal(sm[:], sm[:])
            at = sb.tile([128, HD], F16, tag=f"a16{cc}", name=f"a16{cc}")
            nc.vector.tensor_scalar_mul(at[:], pexp[:], sm[:])
            a16[cc] = at

        # -- stage F: attn^T + F = concat_h(Wv_h @ attn_h^T) --
        at_sb = {}
        for cc in range(2):
            atp = small.tile([HD, 128], F16, tag="ilsmall", name="ilsmall_t")
            nc.tensor.transpose(atp[:], a16[cc][:], wv["ident_sb"])
            at = sb.tile([HD, 128], F16, tag=f"at{cc}", name=f"at{cc}")
            nc.vector.tensor_copy(at[:], atp[:])
            at_sb[cc] = at
        for jc in range(2):
            fp = big.tile([128, C], F32, tag="ilbig", name="ilbig")
            for h in range(H):
                cc, half = divmod(h, 2)
                nc.tensor.matmul(
                    fp[:, h * 64 : (h + 1) * 64],
                    lhsT=wv["wvt_sb"][h][:, jc * 128 : (jc + 1) * 128],
                    rhs=at_sb[cc][:, half * 64 : (half + 1) * 64],
                    start=True,
                    stop=True,
                )
            nc.vector.tensor_copy(F_sb[jc][:], fp[:])


def _kernel(ctx, tc, out, x1, x2, wblob, tmpd, nt, nslab):
    nc = tc.nc
    singles = ctx.enter_context(tc.tile_pool(name="singles", bufs=1))
    SC = SLAB * C  # slab columns (1024)

    # ---- constants / weights ----
    # blob cols: [0:1024] wkq pair, [1024:1536] wq, [1536:2048] wk,
    # [2048:2176] identity, [2176:3200] wvt heads (rows 0-63)
    blob_sb = singles.tile([128, 3200], F16, tag="blob", name="blob")
    # only the identity block is needed during phase A (transposes); the
    # rest of the weights ride the IL1 DMA-idle gap (emitted at end of A)
    nc.scalar.dma_start(out=blob_sb[:, 2048:2176], in_=wblob[:, 2048:2176])
    wv = {
        "wkq_sb": [blob_sb[:, r * 512 : (r + 1) * 512] for r in range(2)],
        "wq_sb": [blob_sb[:, 1024 + r * C : 1024 + (r + 1) * C] for r in range(2)],
        "wk_sb": [blob_sb[:, 1536 + r * C : 1536 + (r + 1) * C] for r in range(2)],
        "ident_sb": blob_sb[:, 2048:2176],
        "wvt_sb": [
            blob_sb[0:HD, 2176 + h * C : 2176 + (h + 1) * C] for h in range(H)
        ],
    }
    tmps_sb = singles.tile([128, 4], F32, tag="tmps", name="tmps")
    nc.scalar.dma_start(out=tmps_sb[:], in_=tmpd[:, :])
    ones_col = singles.tile([128, 1], F16, tag="ones_col", name="ones_col")
    nc.vector.memset(ones_col[:], 1.0)
    ones_row = singles.tile([1, 128], F16, tag="ones_row", name="ones_row")
    nc.vector.memset(ones_row[:], 1.0)
    wv["ones_col"] = ones_col
    wv["ones_row"] = ones_row

    # ---- persistent SBUF ----
    big_pool = ctx.enter_context(tc.tile_pool(name="bigres", bufs=1))
    xT1 = big_pool.tile([128, nslab * SC], F16, tag="xt1", name="xt1")
    y1p = big_pool.tile([128, nt * C], F16, tag="y1p", name="y1p")

    slab_pool = ctx.enter_context(tc.tile_pool(name="slab", bufs=6))
    x16_pool = ctx.enter_context(tc.tile_pool(name="x16", bufs=4))
    xq_pool = ctx.enter_context(tc.tile_pool(name="xq", bufs=4))
    xt2_pool = ctx.enter_context(tc.tile_pool(name="xt2", bufs=4))
    osl_pool = ctx.enter_context(tc.tile_pool(name="osl", bufs=4))

    S_sb = [
        singles.tile([128, 2 * C], F16, tag=f"ssb{s}", name=f"ssb{s}")
        for s in range(2)
    ]
    F_sb = {
        s: [
            singles.tile([128, C], F16, tag=f"f{s}{jc}", name=f"f{s}{jc}")
            for jc in range(2)
        ]
        for s in range(2)
    }

    def load_slab(src, si, alt=False):
        sl = slab_pool.tile([128, SC], F32, tag="slab", name="slab")
        eng = nc.gpsimd if (alt and si % 2 == 1) else nc.sync
        eng.dma_start(
            out=sl[:].rearrange("p (t c) -> p t c", t=SLAB),
            in_=src[si * SLAB * TT : (si + 1) * SLAB * TT, :].rearrange(
                "(p t) c -> p t c", p=128
            ),
        )
        return sl

    # ================= phase A: stream x1 =================
    with ExitStack() as pa:
        psS = pa.enter_context(tc.tile_pool(name="psS1", bufs=1, space="PSUM"))
        psT = pa.enter_context(tc.tile_pool(name="psT1", bufs=2, space="PSUM"))
        S_ps = psS.tile([128, 2 * C], F32, tag="s1", name="s1")
        for si in range(nslab):
            sl = load_slab(x1, si)
            x16 = x16_pool.tile([128, SC], F16, tag="x16", name="x16")
            nc.vector.tensor_copy(x16[:], sl[:])
            for t in range(SLAB):
                ti = si * SLAB + t
                x_t = x16[:, t * C : (t + 1) * C]
                for c0 in range(2):
                    nc.tensor.matmul(
                        S_ps[:, c0 * C : (c0 + 1) * C],
                        lhsT=x_t[:, c0 * 128 : (c0 + 1) * 128],
                        rhs=x_t,
                        start=(ti == 0),
                        stop=(ti == nt - 1),
                        skip_group_check=True,
                    )
            tp = psT.tile([128, SC], F16, tag="tp1", name="tp1")
            for t in range(SLAB):
                for c0 in range(2):
                    nc.tensor.transpose(
                        tp[:, t * C + c0 * 128 : t * C + (c0 + 1) * 128],
                        x16[:, t * C + c0 * 128 : t * C + (c0 + 1) * 128],
                        wv["ident_sb"],
                    )
            nc.scalar.activation(
                xT1[:, si * SC : (si + 1) * SC], tp[:], AF.Copy
            )
        nc.vector.tensor_copy(S_sb[0][:], S_ps[:])
        # deferred weight loads: transfer after the last x1 slab, ready
        # before IL1's first matmul consumes them
        nc.sync.dma_start(out=blob_sb[:, 0:2048], in_=wblob[:, 0:2048])
        nc.sync.dma_start(out=blob_sb[:, 2176:3200], in_=wblob[:, 2176:3200])

    # ================= interlude 1 =================
    _interlude(tc, singles, wv, S_sb[0], tmps_sb[:, 0:2], F_sb[0])

    # ================= phase B: stream x2 =================
    with ExitStack() as pb:
        psS = pb.enter_context(tc.tile_pool(name="psS2", bufs=1, space="PSUM"))
        psT = pb.enter_context(tc.tile_pool(name="psT2", bufs=2, space="PSUM"))
        psO = pb.enter_context(tc.tile_pool(name="psO2", bufs=2, space="PSUM"))
        S_ps = psS.tile([128, 2 * C], F32, tag="s2", name="s2")
        pending = []  # (xt2 tile, si) queue; P2p runs two slabs behind

        def emit_p2p(xt2_t, si_t):
            op = psO.tile([128, SC], F32, tag="opB", name="opB")
            for t in range(SLAB):
                for c0 in range(2):
                    nc.tensor.matmul(
                        op[:, t * C : (t + 1) * C],
                        lhsT=xt2_t[:, t * C + c0 * 128 : t * C + (c0 + 1) * 128],
                        rhs=F_sb[0][c0][:],
                        start=(c0 == 0),
                        stop=(c0 == 1),
                        skip_group_check=True,
                    )
            nc.scalar.activation(
                y1p[:, si_t * SC : (si_t + 1) * SC], op[:], AF.Copy
            )

        for si in range(nslab):
            sl = load_slab(x2, si)
            if len(pending) >= 2:
                emit_p2p(*pending.pop(0))
            x16 = x16_pool.tile([128, SC], F16, tag="x16", name="x16")
            nc.vector.tensor_copy(x16[:], sl[:])
            # fp8 copy in chunk-major layout [p, (c, t, n)] so DoubleRow
            # slot views [p, 2, 128] are fully contiguous (HW requirement)
            xq = xq_pool.tile([128, SC], FP8, tag="xq", name="xq")
            nc.gpsimd.tensor_copy(
                xq[:].rearrange("p (c t n) -> p t c n", c=2, t=SLAB),
                sl[:].rearrange("p (t c n) -> p t c n", t=SLAB, c=2),
            )
            tp = psT.tile([128, SC], F16, tag="tp2", name="tp2")
            for t in range(SLAB):
                for c0 in range(2):
                    nc.tensor.transpose(
                        tp[:, t * C + c0 * 128 : t * C + (c0 + 1) * 128],
                        x16[:, t * C + c0 * 128 : t * C + (c0 + 1) * 128],
                        wv["ident_sb"],
                    )
            xt2 = xt2_pool.tile([128, SC], F16, tag="xt2", name="xt2")
            nc.vector.tensor_copy(xt2[:], tp[:])
            xq_v = xq[:].rearrange("p (c t n) -> p c t n", c=2, t=SLAB)
            for dd in range(SLAB // 2):
                d = si * (SLAB // 2) + dd
                for c0 in range(2):
                    for c1 in range(2):
                        nc.tensor.matmul(
                            S_ps[
                                :,
                                c0 * C + c1 * 128 : c0 * C + (c1 + 1) * 128,
                            ],
                            lhsT=xq_v[:, c0, 2 * dd : 2 * dd + 2, :],
                            rhs=xq_v[:, c1, 2 * dd : 2 * dd + 2, :],
                            start=(d == 0),
                            stop=(d == nslab * (SLAB // 2) - 1),
                            perf_mode=DR,
                            skip_group_check=True,
                        )
            pending.append((xt2, si))
        for item in pending:
            emit_p2p(*item)
        nc.vector.tensor_copy(S_sb[1][:], S_ps[:])

    # ================= interlude 2 =================
    _interlude(tc, singles, wv, S_sb[1], tmps_sb[:, 2:4], F_sb[1])

    # ================= phase C: out = y1p + x1 @ F2 =================
    with ExitStack() as pc:
        psO = pc.enter_context(tc.tile_pool(name="psOC", bufs=4, space="PSUM"))
        HS = SC // 2  # half-slab columns
        for si in range(nslab):
            osl = osl_pool.tile([128, SC], F16, tag="osl", name="osl")
            # half-slab pipelining: matmuls -> add -> (after both) DMA
            op = psO.tile([128, SC], F32, tag="opC", name="opC")
            for t in range(SLAB):
                base = si * SC + t * C
                ht = t >= SLAB // 2
                if ht:
                    # back half: seed the accumulation group with y1p via an
                    # identity matmul (exact-region, group-opening), so the
                    # idle ACT engine can do the copy-out instead of DVE
                    nc.tensor.matmul(
                        op[:, t * C : (t + 1) * C],
                        lhsT=wv["ident_sb"],
                        rhs=y1p[:, si * SC + t * C : si * SC + (t + 1) * C],
                        start=True,
                        stop=False,
                        skip_group_check=True,
                    )
                for c0 in range(2):
                    nc.tensor.matmul(
                        op[:, t * C : (t + 1) * C],
                        lhsT=xT1[:, base + c0 * 128 : base + (c0 + 1) * 128],
                        rhs=F_sb[1][c0][:],
                        start=(c0 == 0) and not ht,
                        stop=(c0 == 1),
                        skip_group_check=True,
                    )
            HS = SC // 2
            nc.vector.tensor_tensor(
                out=osl[:, 0:HS],
                in0=op[:, 0:HS],
                in1=y1p[:, si * SC : si * SC + HS],
                op=mybir.AluOpType.add,
            )
            nc.scalar.activation(osl[:, HS:SC], op[:, HS:SC], AF.Copy)
            nc.sync.dma_start(
                out=out[si * SLAB * TT : (si + 1) * SLAB * TT, :].rearrange(
                    "(p t) c -> p t c", p=128
                ),
                in_=osl[:].rearrange("p (t c) -> p t c", t=SLAB),
            )


def _host_prep(w_qkv, temperature, temperature2):
    w = np.asarray(w_qkv, dtype=np.float32)
    wq = w[:, 0:C].astype(np.float16)
    wk = w[:, C : 2 * C].astype(np.float16)
    wvt = np.ascontiguousarray(w[:, 2 * C : 3 * C].T.reshape(H, HD, C)).astype(
        np.float16
    )
    blob = np.zeros((128, 3200), dtype=np.float16)
    for r in range(2):
        blob[:, r * 512 : r * 512 + C] = wk[r * 128 : (r + 1) * 128, :]
        blob[:, r * 512 + C : (r + 1) * 512] = wq[r * 128 : (r + 1) * 128, :]
        blob[:, 1024 + r * C : 1024 + (r + 1) * C] = wq[r * 128 : (r + 1) * 128, :]
        blob[:, 1536 + r * C : 1536 + (r + 1) * C] = wk[r * 128 : (r + 1) * 128, :]
    blob[:, 2048:2176] = np.eye(128, dtype=np.float16)
    for h in range(H):
        blob[0:HD, 2176 + h * C : 2176 + (h + 1) * C] = wvt[h]
    tmp = []
    for tarr in (temperature, temperature2):
        t = np.log(np.asarray(tarr, dtype=np.float32).reshape(H))
        tmp.append(
            np.stack(
                [np.repeat(t[[0, 1]], 64), np.repeat(t[[2, 3]], 64)], axis=1
            ).astype(np.float32)
        )
    tmpd = np.concatenate(tmp, axis=1).astype(np.float32)
    return blob, tmpd


_NC_CACHE = {}
LAST_RESULT = None


def _get_nc(n_tokens):
    if n_tokens not in _NC_CACHE:
        _NC_CACHE[n_tokens] = _build(n_tokens)
    return _NC_CACHE[n_tokens]


def kernel(x1, x2, w_qkv, temperature, temperature2):
    global LAST_RESULT
    x1 = np.asarray(x1, dtype=np.float32)
    x2 = np.asarray(x2, dtype=np.float32)
    b, n, c = x1.shape
    assert c == C and b == NCORES, (b, n, c)
    wblob, tmpd = _host_prep(w_qkv, temperature, temperature2)
    nc = _get_nc(n)
    in_maps = [
        {
            "x1": np.ascontiguousarray(x1[i]),
            "x2": np.ascontiguousarray(x2[i]),
            "wblob": wblob,
            "tmpd": tmpd,
        }
        for i in range(NCORES)
    ]
    res = run_bass_kernel_spmd(nc, in_maps, list(range(NCORES)))
    LAST_RESULT = res
    return (
        np.stack([np.asarray(r["out"]) for r in res.results])
        .astype(np.float32)
        .reshape(b, n, c)
    )


# revision 44
# speedup vs baseline: 1.0020x; 1.0020x over previous
"""JumpAttention (channel attention, cross-swapped values) on 8 trn2 cores.

v3: 3-phase streamed schedule, pure data-parallel over B (B=8 == n_cores).

Math (per batch, per head h, hd=64):
  G_h  = q_h k_h^T (contraction over N) == Wq_h^T (x^T x) Wk_h
  attn = softmax(G / (||q|| ||k||) * temp, axis=d)
  y1+y2 = x2 @ F1 + x1 @ F2,  F_s = concat_h(Wv_h @ attn_s_h^T)

Schedule:
  Phase A: stream x1 slabs (contiguous per-partition descriptors,
           "(p t) c"): cast f16, S1 += x1^T x1, PE-transpose into
           resident xT1 (f16).
  IL1:     attn1 from S1 -> F1 (f16). rsqrt via exp(-.5 ln n) so the
           ACT engine only ever needs the {copy, ln, exp} table.
  Phase B: stream x2 slabs: cast f16 + fp8; S2 via fp8 DoubleRow
           matmuls; transpose -> xt2 (transient); y1 partial
           = x2 @ F1 -> resident y1p (f16). x2 is never stored.
           x2@F1 matmuls run two slabs behind their transposes so the
           PE never round-trips through the copy engines.
  IL2:     attn2 from S2 -> F2
  Phase C: out = y1p + x1 @ F2 (from xT1), f16 out, host upcasts.

All DMAs use the "(p t) c" token permutation (partition-major contiguous
rows, 4KB/2KB descriptors); input and output use the same permutation so
it cancels. S sums are token-order invariant.
"""

import os
import sys
from contextlib import ExitStack

import numpy as np
import ml_dtypes

for _p in ("/opt/trn_rl_repo",):
    if _p not in sys.path and os.path.isdir(_p):
        sys.path.insert(0, _p)

import concourse.bass as bass  # noqa: E402
import concourse.tile as tile  # noqa: E402
from concourse import bacc, mybir  # noqa: E402
from concourse.bass_utils import run_bass_kernel_spmd  # noqa: E402

B, N_FULL, C = 8, 16384, 256
H, HD = 4, 64
NCORES = 8
TT = 128
SLAB = 4

F32 = mybir.dt.float32
F16 = mybir.dt.float16
FP8 = mybir.dt.float8e4
AF = mybir.ActivationFunctionType
DR = mybir.MatmulPerfMode.DoubleRow
EPS2 = 1e-24  # clamp on squared norms ~ (eps=1e-12)**2


def _build(n_tokens: int):
    nt = n_tokens // TT
    nslab = n_tokens // (TT * SLAB)
    nc = bacc.Bacc(
        "TRN2", target_bir_lowering=False, debug=False, num_devices=NCORES
    )
    x1 = nc.dram_tensor("x1", [n_tokens, C], F32, kind="ExternalInput").ap()
    x2 = nc.dram_tensor("x2", [n_tokens, C], F32, kind="ExternalInput").ap()
    wblob = nc.dram_tensor("wblob", [128, 3200], F16, kind="ExternalInput").ap()
    tmpd = nc.dram_tensor("tmpd", [128, 4], F32, kind="ExternalInput").ap()
    out = nc.dram_tensor("out", [n_tokens, C], F16, kind="ExternalOutput").ap()

    with tile.TileContext(nc) as tc, ExitStack() as ctx:
        _kernel(ctx, tc, out, x1, x2, wblob, tmpd, nt, nslab)
    nc.compile()
    return nc


def _interlude(tc, sb, wv, S_sb, lntmp_cc, F_sb):
    """One stream: S (f16 [128, 2C], chunks side by side) -> F (f16, 2 chunks).

    lntmp_cc: [128, 2] f32, per-cc ln(temperature) striped per partition.
    Only Copy/Ln/Exp are used on ACT (single function table).
    """
    nc = tc.nc
    with ExitStack() as il:
        big = il.enter_context(tc.tile_pool(name="ilbig", bufs=6, space="PSUM"))
        small = il.enter_context(tc.tile_pool(name="ilsmall", bufs=2, space="PSUM"))

        Sv = [S_sb[:, jc * C : (jc + 1) * C] for jc in range(2)]

        # -- stage A: [T_k | T_q] = S @ [Wk | Wq] --
        t_sb = {}
        for ic in range(2):
            tp = big.tile([128, 2 * C], F32, tag="ilbig", name="ilbig")
            for jc in range(2):
                nc.tensor.matmul(
                    tp[:],
                    lhsT=Sv[jc][:, ic * 128 : (ic + 1) * 128],
                    rhs=wv["wkq_sb"][jc],
                    start=(jc == 0),
                    stop=(jc == 1),
                )
            st = sb.tile([128, 2 * C], F16, tag=f"t{ic}", name=f"t{ic}")
            nc.vector.tensor_copy(st[:, 0:C], tp[:, 0:C])
            nc.vector.tensor_copy(st[:, C : 2 * C], tp[:, C : 2 * C])
            t_sb["k", ic] = st[:, 0:C]
            t_sb["q", ic] = st[:, C : 2 * C]

        # -- stage B: G = Wq^T @ T_k ; U = W * T --
        g_ps = {}
        for cc in range(2):
            g = big.tile([128, C], F32, tag="ilbig", name="ilbig")
            for ic in range(2):
                nc.tensor.matmul(
                    g[:],
                    lhsT=wv["wq_sb"][ic][:, cc * 128 : (cc + 1) * 128],
                    rhs=t_sb["k", ic],
                    start=(ic == 0),
                    stop=(ic == 1),
                )
            g_ps[cc] = g
        u_sb = {}
        for nm in ("q", "k"):
            w_sb = wv["wq_sb"] if nm == "q" else wv["wk_sb"]
            for ic in range(2):
                u = sb.tile([128, C], F16, tag=f"u{nm}{ic}", name=f"u{nm}{ic}")
                nc.vector.tensor_mul(u[:], w_sb[ic], t_sb[nm, ic])
                u_sb[nm, ic] = u

        # -- stage C: squared norms + rsqrt via exp(-.5 ln n) --
        nq_ps = {}
        for cc in range(2):
            nq = small.tile([128, 1], F32, tag="ilsmall", name="ilsmall")
            for ic in range(2):
                nc.tensor.matmul(
                    nq[:],
                    lhsT=u_sb["q", ic][:, cc * 128 : (cc + 1) * 128],
                    rhs=wv["ones_col"][:],
                    start=(ic == 0),
                    stop=(ic == 1),
                )
            nq_ps[cc] = nq
        nk = small.tile([1, C], F32, tag="ilsmall", name="ilsmall_r")
        for ic in range(2):
            nc.tensor.matmul(
                nk[:],
                lhsT=wv["ones_col"][:],
                rhs=u_sb["k", ic],
                start=(ic == 0),
                stop=(ic == 1),
            )
        # cluster all Ln ops, then all Exp ops: exactly two ACT table loads
        lq = {}
        for cc in range(2):
            l_ = sb.tile([128, 1], F32, tag=f"lq{cc}", name=f"lq{cc}")
            nc.vector.tensor_scalar_max(l_[:], nq_ps[cc][:], EPS2)
            lq[cc] = l_
        lk = sb.tile([1, C], F32, tag="lk", name="lk")
        nc.vector.tensor_scalar_max(lk[:], nk[:], EPS2)
        for cc in range(2):
            nc.scalar.activation(lq[cc][:], lq[cc][:], AF.Ln)
        nc.scalar.activation(lk[:], lk[:], AF.Ln)
        invq = {}
        for cc in range(2):
            iq = sb.tile([128, 1], F32, tag=f"invq{cc}", name=f"invq{cc}")
            # invq = exp(-.5 ln nq + ln temp) = temp / ||q||
            nc.scalar.activation(
                iq[:], lq[cc][:], AF.Exp, scale=-0.5,
                bias=lntmp_cc[:, cc : cc + 1],
            )
            invq[cc] = iq
        ik16 = sb.tile([1, C], F16, tag="invk16", name="invk16")
        nc.scalar.activation(ik16[:], lk[:], AF.Exp, scale=-0.5)

        # -- stage D: invk broadcast + logits + row max --
        bc = big.tile([128, C], F32, tag="ilbig", name="ilbig")
        nc.tensor.matmul(
            bc[:], lhsT=wv["ones_row"][:], rhs=ik16[:], start=True, stop=True
        )
        lp_sb, nm_sb = {}, {}
        for cc in range(2):
            lp = sb.tile([128, HD], F32, tag=f"lp{cc}", name=f"lp{cc}")
            for half in range(2):
                h = 2 * cc + half
                rs = slice(half * 64, (half + 1) * 64)
                cs = slice(h * 64, (h + 1) * 64)
                nc.vector.tensor_scalar_mul(
                    lp[rs, :], g_ps[cc][rs, cs], invq[cc][rs, :]
                )
                nc.vector.tensor_mul(lp[rs, :], lp[rs, :], bc[rs, cs])
            nmt = sb.tile([128, 1], F32, tag=f"nm{cc}", name=f"nm{cc}")
            nc.vector.tensor_reduce(
                nmt[:], lp[:], axis=mybir.AxisListType.X,
                op=mybir.AluOpType.max, negate=True,
            )
            lp_sb[cc] = lp
            nm_sb[cc] = nmt

        # -- stage E: Exp + normalize --
        a16 = {}
        for cc in range(2):
            pexp = sb.tile([128, HD], F32, tag=f"pexp{cc}", name=f"pexp{cc}")
            sm = sb.tile([128, 1], F32, tag=f"sm{cc}", name=f"sm{cc}")
            nc.scalar.activation(
                pexp[:], lp_sb[cc][:], AF.Exp,
                bias=nm_sb[cc][:], accum_out=sm[:],
            )
            a16[cc] = (pexp, sm)
        for cc in range(2):
            pexp, sm = a16[cc]
            nc.vector.reciprocal(sm[:], sm[:])
            at = sb.tile([128, HD], F16, tag=f"a16{cc}", name=f"a16{cc}")
            nc.vector.tensor_scalar_mul(at[:], pexp[:], sm[:])
            a16[cc] = at

        # -- stage F: attn^T + F = concat_h(Wv_h @ attn_h^T) --
        at_sb = {}
        for cc in range(2):
            atp = small.tile([HD, 128], F16, tag="ilsmall", name="ilsmall_t")
            nc.tensor.transpose(atp[:], a16[cc][:], wv["ident_sb"])
            at = sb.tile([HD, 128], F16, tag=f"at{cc}", name=f"at{cc}")
            nc.vector.tensor_copy(at[:], atp[:])
            at_sb[cc] = at
        for jc in range(2):
            fp = big.tile([128, C], F32, tag="ilbig", name="ilbig")
            for h in range(H):
                cc, half = divmod(h, 2)
                nc.tensor.matmul(
                    fp[:, h * 64 : (h + 1) * 64],
                    lhsT=wv["wvt_sb"][h][:, jc * 128 : (jc + 1) * 128],
                    rhs=at_sb[cc][:, half * 64 : (half + 1) * 64],
                    start=True,
                    stop=True,
                )
            nc.vector.tensor_copy(F_sb[jc][:], fp[:])


def _kernel(ctx, tc, out, x1, x2, wblob, tmpd, nt, nslab):
    nc = tc.nc
    singles = ctx.enter_context(tc.tile_pool(name="singles", bufs=1))
    SC = SLAB * C  # slab columns (1024)

    # ---- constants / weights ----
    # blob cols: [0:1024] wkq pair, [1024:1536] wq, [1536:2048] wk,
    # [2048:2176] identity, [2176:3200] wvt heads (rows 0-63)
    blob_sb = singles.tile([128, 3200], F16, tag="blob", name="blob")
    # only the identity block is needed during phase A (transposes); the
    # rest of the weights ride the IL1 DMA-idle gap (emitted at end of A)
    nc.scalar.dma_start(out=blob_sb[:, 2048:2176], in_=wblob[:, 2048:2176])
    wv = {
        "wkq_sb": [blob_sb[:, r * 512 : (r + 1) * 512] for r in range(2)],
        "wq_sb": [blob_sb[:, 1024 + r * C : 1024 + (r + 1) * C] for r in range(2)],
        "wk_sb": [blob_sb[:, 1536 + r * C : 1536 + (r + 1) * C] for r in range(2)],
        "ident_sb": blob_sb[:, 2048:2176],
        "wvt_sb": [
            blob_sb[0:HD, 2176 + h * C : 2176 + (h + 1) * C] for h in range(H)
        ],
    }
    tmps_sb = singles.tile([128, 4], F32, tag="tmps", name="tmps")
    nc.scalar.dma_start(out=tmps_sb[:], in_=tmpd[:, :])
    ones_col = singles.tile([128, 1], F16, tag="ones_col", name="ones_col")
    nc.vector.memset(ones_col[:], 1.0)
    ones_row = singles.tile([1, 128], F16, tag="ones_row", name="ones_row")
    nc.vector.memset(ones_row[:], 1.0)
    wv["ones_col"] = ones_col
    wv["ones_row"] = ones_row

    # ---- persistent SBUF ----
    big_pool = ctx.enter_context(tc.tile_pool(name="bigres", bufs=1))
    xT1 = big_pool.tile([128, nslab * SC], F16, tag="xt1", name="xt1")
    y1p = big_pool.tile([128, nt * C], F16, tag="y1p", name="y1p")

    slab_pool = ctx.enter_context(tc.tile_pool(name="slab", bufs=6))
    x16_pool = ctx.enter_context(tc.tile_pool(name="x16", bufs=4))
    xq_pool = ctx.enter_context(tc.tile_pool(name="xq", bufs=4))
    xt2_pool = ctx.enter_context(tc.tile_pool(name="xt2", bufs=4))
    osl_pool = ctx.enter_context(tc.tile_pool(name="osl", bufs=4))

    S_sb = [
        singles.tile([128, 2 * C], F16, tag=f"ssb{s}", name=f"ssb{s}")
        for s in range(2)
    ]
    F_sb = {
        s: [
            singles.tile([128, C], F16, tag=f"f{s}{jc}", name=f"f{s}{jc}")
            for jc in range(2)
        ]
        for s in range(2)
    }

    def load_slab(src, si, alt=False):
        sl = slab_pool.tile([128, SC], F32, tag="slab", name="slab")
        eng = nc.gpsimd if (alt and si % 2 == 1) else nc.sync
        eng.dma_start(
            out=sl[:].rearrange("p (t c) -> p t c", t=SLAB),
            in_=src[si * SLAB * TT : (si + 1) * SLAB * TT, :].rearrange(
                "(p t) c -> p t c", p=128
            ),
        )
        return sl

    # ================= phase A: stream x1 =================
    with ExitStack() as pa:
        psS = pa.enter_context(tc.tile_pool(name="psS1", bufs=1, space="PSUM"))
        psT = pa.enter_context(tc.tile_pool(name="psT1", bufs=2, space="PSUM"))
        S_ps = psS.tile([128, 2 * C], F32, tag="s1", name="s1")
        for si in range(nslab):
            sl = load_slab(x1, si)
            x16 = x16_pool.tile([128, SC], F16, tag="x16", name="x16")
            nc.vector.tensor_copy(x16[:], sl[:])
            for t in range(SLAB):
                ti = si * SLAB + t
                x_t = x16[:, t * C : (t + 1) * C]
                for c0 in range(2):
                    nc.tensor.matmul(
                        S_ps[:, c0 * C : (c0 + 1) * C],
                        lhsT=x_t[:, c0 * 128 : (c0 + 1) * 128],
                        rhs=x_t,
                        start=(ti == 0),
                        stop=(ti == nt - 1),
                        skip_group_check=True,
                    )
            tp = psT.tile([128, SC], F16, tag="tp1", name="tp1")
            for t in range(SLAB):
                for c0 in range(2):
                    nc.tensor.transpose(
                        tp[:, t * C + c0 * 128 : t * C + (c0 + 1) * 128],
                        x16[:, t * C + c0 * 128 : t * C + (c0 + 1) * 128],
                        wv["ident_sb"],
                    )
            nc.scalar.activation(
                xT1[:, si * SC : (si + 1) * SC], tp[:], AF.Copy
            )
        nc.vector.tensor_copy(S_sb[0][:], S_ps[:])
        # deferred weight loads: transfer after the last x1 slab, ready
        # before IL1's first matmul consumes them
        nc.sync.dma_start(out=blob_sb[:, 0:2048], in_=wblob[:, 0:2048])
        nc.sync.dma_start(out=blob_sb[:, 2176:3200], in_=wblob[:, 2176:3200])

    # ================= interlude 1 =================
    _interlude(tc, singles, wv, S_sb[0], tmps_sb[:, 0:2], F_sb[0])

    # ================= phase B: stream x2 =================
    with ExitStack() as pb:
        psS = pb.enter_context(tc.tile_pool(name="psS2", bufs=1, space="PSUM"))
        psT = pb.enter_context(tc.tile_pool(name="psT2", bufs=2, space="PSUM"))
        psO = pb.enter_context(tc.tile_pool(name="psO2", bufs=2, space="PSUM"))
        S_ps = psS.tile([128, 2 * C], F32, tag="s2", name="s2")
        pending = []  # (xt2 tile, si) queue; P2p runs two slabs behind

        def emit_p2p(xt2_t, si_t):
            op = psO.tile([128, SC], F32, tag="opB", name="opB")
            for t in range(SLAB):
                for c0 in range(2):
                    nc.tensor.matmul(
                        op[:, t * C : (t + 1) * C],
                        lhsT=xt2_t[:, t * C + c0 * 128 : t * C + (c0 + 1) * 128],
                        rhs=F_sb[0][c0][:],
                        start=(c0 == 0),
                        stop=(c0 == 1),
                        skip_group_check=True,
                    )
            nc.scalar.activation(
                y1p[:, si_t * SC : (si_t + 1) * SC], op[:], AF.Copy
            )

        for si in range(nslab):
            sl = load_slab(x2, si)
            if len(pending) >= 2:
                emit_p2p(*pending.pop(0))
            x16 = x16_pool.tile([128, SC], F16, tag="x16", name="x16")
            nc.vector.tensor_copy(x16[:], sl[:])
            # fp8 copy in chunk-major layout [p, (c, t, n)] so DoubleRow
            # slot views [p, 2, 128] are fully contiguous (HW requirement)
            xq = xq_pool.tile([128, SC], FP8, tag="xq", name="xq")
            nc.gpsimd.tensor_copy(
                xq[:].rearrange("p (c t n) -> p t c n", c=2, t=SLAB),
                sl[:].rearrange("p (t c n) -> p t c n", t=SLAB, c=2),
            )
            tp = psT.tile([128, SC], F16, tag="tp2", name="tp2")
            for t in range(SLAB):
                for c0 in range(2):
                    nc.tensor.transpose(
                        tp[:, t * C + c0 * 128 : t * C + (c0 + 1) * 128],
                        x16[:, t * C + c0 * 128 : t * C + (c0 + 1) * 128],
                        wv["ident_sb"],
                    )
            xt2 = xt2_pool.tile([128, SC], F16, tag="xt2", name="xt2")
            nc.vector.tensor_copy(xt2[:], tp[:])
            xq_v = xq[:].rearrange("p (c t n) -> p c t n", c=2, t=SLAB)
            for dd in range(SLAB // 2):
                d = si * (SLAB // 2) + dd
                for c0 in range(2):
                    for c1 in range(2):
                        nc.tensor.matmul(
                            S_ps[
                                :,
                                c0 * C + c1 * 128 : c0 * C + (c1 + 1) * 128,
                            ],
                            lhsT=xq_v[:, c0, 2 * dd : 2 * dd + 2, :],
                            rhs=xq_v[:, c1, 2 * dd : 2 * dd + 2, :],
                            start=(d == 0),
                            stop=(d == nslab * (SLAB // 2) - 1),
                            perf_mode=DR,
                            skip_group_check=True,
                        )
            pending.append((xt2, si))
        for item in pending:
            emit_p2p(*item)
        nc.vector.tensor_copy(S_sb[1][:], S_ps[:])

    # ================= interlude 2 =================
    _interlude(tc, singles, wv, S_sb[1], tmps_sb[:, 2:4], F_sb[1])

    # ================= phase C: out = y1p + x1 @ F2 =================
    with ExitStack() as pc:
        psO = pc.enter_context(tc.tile_pool(name="psOC", bufs=4, space="PSUM"))
        HS = SC // 2  # half-slab columns
        for si in range(nslab):
            osl = osl_pool.tile([128, SC], F16, tag="osl", name="osl")
            # half-slab pipelining: matmuls -> add -> (after both) DMA
            op = psO.tile([128, SC], F32, tag="opC", name="opC")
            for t in range(SLAB):
                base = si * SC + t * C
                for c0 in range(2):
                    nc.tensor.matmul(
                        op[:, t * C : (t + 1) * C],
                        lhsT=xT1[:, base + c0 * 128 : base + (c0 + 1) * 128],
                        rhs=F_sb[1][c0][:],
                        start=(c0 == 0),
                        stop=(c0 == 1),
                        skip_group_check=True,
                    )
            nc.vector.tensor_tensor(
                out=osl[:],
                in0=op[:],
                in1=y1p[:, si * SC : (si + 1) * SC],
                op=mybir.AluOpType.add,
            )
            nc.sync.dma_start(
                out=out[si * SLAB * TT : (si + 1) * SLAB * TT, :].rearrange(
                    "(p t) c -> p t c", p=128
                ),
                in_=osl[:].rearrange("p (t c) -> p t c", t=SLAB),
            )


def _host_prep(w_qkv, temperature, temperature2):
    w = np.asarray(w_qkv, dtype=np.float32)
    wq = w[:, 0:C].astype(np.float16)
    wk = w[:, C : 2 * C].astype(np.float16)
    wvt = np.ascontiguousarray(w[:, 2 * C : 3 * C].T.reshape(H, HD, C)).astype(
        np.float16
    )
    blob = np.zeros((128, 3200), dtype=np.float16)
    for r in range(2):
        blob[:, r * 512 : r * 512 + C] = wk[r * 128 : (r + 1) * 128, :]
        blob[:, r * 512 + C : (r + 1) * 512] = wq[r * 128 : (r + 1) * 128, :]
        blob[:, 1024 + r * C : 1024 + (r + 1) * C] = wq[r * 128 : (r + 1) * 128, :]
        blob[:, 1536 + r * C : 1536 + (r + 1) * C] = wk[r * 128 : (r + 1) * 128, :]
    blob[:, 2048:2176] = np.eye(128, dtype=np.float16)
    for h in range(H):
        blob[0:HD, 2176 + h * C : 2176 + (h + 1) * C] = wvt[h]
    tmp = []
    for tarr in (temperature, temperature2):
        t = np.log(np.asarray(tarr, dtype=np.float32).reshape(H))
        tmp.append(
            np.stack(
                [np.repeat(t[[0, 1]], 64), np.repeat(t[[2, 3]], 64)], axis=1
            ).astype(np.float32)
        )
    tmpd = np.concatenate(tmp, axis=1).astype(np.float32)
    return blob, tmpd


_NC_CACHE = {}
LAST_RESULT = None


def _get_nc(n_tokens):
    if n_tokens not in _NC_CACHE:
        _NC_CACHE[n_tokens] = _build(n_tokens)
    return _NC_CACHE[n_tokens]


def kernel(x1, x2, w_qkv, temperature, temperature2):
    global LAST_RESULT
    x1 = np.asarray(x1, dtype=np.float32)
    x2 = np.asarray(x2, dtype=np.float32)
    b, n, c = x1.shape
    assert c == C and b == NCORES, (b, n, c)
    wblob, tmpd = _host_prep(w_qkv, temperature, temperature2)
    nc = _get_nc(n)
    in_maps = [
        {
            "x1": np.ascontiguousarray(x1[i]),
            "x2": np.ascontiguousarray(x2[i]),
            "wblob": wblob,
            "tmpd": tmpd,
        }
        for i in range(NCORES)
    ]
    res = run_bass_kernel_spmd(nc, in_maps, list(range(NCORES)))
    LAST_RESULT = res
    return (
        np.stack([np.asarray(r["out"]) for r in res.results])
        .astype(np.float32)
        .reshape(b, n, c)
    )
